# revision 9
# baseline (speedup 1.0000x reference)
"""Trainium2 Bass kernel for nn_DotAttention (dense_transformer).

Reference computation (per batch b):
    proj   = enc_b @ W_enc + b_enc            # (S, H)
    scores = proj @ dh_b                      # (S,)
    w      = softmax(scores)                  # (S,)
    ctx    = proj^T @ w                       # (H,)
    out    = tanh([ctx; dh_b] @ W_ctx + b_ctx)

Algebraic restructuring (exact in real arithmetic):
    scores = enc_b @ (W_enc @ dh_b) + (b_enc . dh_b)
             -- the scalar offset cancels in softmax, so drop it
    ctx    = W_enc^T @ (enc_b^T @ w) + b_enc  -- since sum(w) == 1
This avoids materializing proj (275 GFLOP -> ~1.5 GFLOP) and makes the
kernel one streaming pass over encoder_outputs, fused per 128-row tile:
DVE tensor_tensor_reduce for scores, ACT exp, PE fp32r matmuls for the
weighted sum.

Softmax max-trick uses a precomputed safe bound m = 3.9*||v||
(v = W_enc @ dh_b): scores are N(0, ||v||^2) for the given input
distribution, so the true max lies within [2.4, 6]*sigma with
overwhelming probability, keeping every exp/sum/reciprocal in normal
fp32 range; softmax is shift-invariant so the result is exact.

float32r (TF32-class PE path, 1 cycle/row vs 4 for fp32; measured HW
matmul relerr ~1.3e-4) is used for the weighted-sum pass and the tail
projections. The scores themselves are computed in full fp32 on DVE.

Sharding: data-parallel over batch B=32 across 8 cores (4 per core);
weights replicated.
"""

import numpy as np

P = 128           # SBUF partitions
NCORES = 8
ALPHA = 3.9       # softmax max-bound multiplier (see module docstring)

_BUILD_CACHE = {}


def _build_nc(NB, S, F, H):
    """Build the per-core program. NB: batches per core; S: sequence;
    F = 2H: encoder feature dim; H: hidden dim."""
    import concourse.bacc as bacc
    import concourse.tile as tile
    from concourse import mybir
    from concourse.masks import make_identity

    f32 = mybir.dt.float32
    f32r = mybir.dt.float32r
    ALU = mybir.AluOpType
    ACTF = mybir.ActivationFunctionType
    X = mybir.AxisListType.X

    NS = S // P          # s-chunks per batch
    NFC = F // P         # f-chunks (128-wide)
    NH = H // P          # h-chunks (128-wide)
    NMM = min(512, F)    # psum bank width in fp32 (moving-dim chunk)
    NMH = min(512, H)    # tail column chunk
    NJ = F // NMM        # pass-2 column chunks
    NHJ = H // NMH       # tail column chunks
    assert NFC == 2 * NH

    nc = bacc.Bacc("TRN2", target_bir_lowering=False, debug=False)

    enc_d = nc.declare_dram_parameter("enc", [NB, S, F], f32, isOutput=False)
    dh_d = nc.declare_dram_parameter("dh", [NB, H], f32, isOutput=False)
    wenc_d = nc.declare_dram_parameter("w_enc", [F, H], f32, isOutput=False)
    benc_d = nc.declare_dram_parameter("b_enc", [H], f32, isOutput=False)
    wctx_d = nc.declare_dram_parameter("w_ctx", [F, H], f32, isOutput=False)
    bctx_d = nc.declare_dram_parameter("b_ctx", [H], f32, isOutput=False)
    out_d = nc.declare_dram_parameter("out", [NB, H], f32, isOutput=True)
    attn_d = nc.declare_dram_parameter("attn", [NB, S], f32, isOutput=True)

    # DRAM scratch for cheap layout changes (small)
    v_scr = nc.dram_tensor("v_scr", [NB, F], f32)
    nm_scr = nc.dram_tensor("nm_scr", [NB, 1], f32)
    u_scr = nc.dram_tensor("u_scr", [NB, F], f32r)
    ctx_scr = nc.dram_tensor("ctx_scr", [NB, H], f32r)

    enc = enc_d.ap()
    dh = dh_d.ap()
    wenc = wenc_d.ap()
    wctx = wctx_d.ap()

    with tile.TileContext(nc) as tc:
        with (
            tc.tile_pool(name="const", bufs=1) as const,
            tc.tile_pool(name="encp", bufs=8) as encp,
            tc.tile_pool(name="vbc", bufs=2) as vbc,
            tc.tile_pool(name="sm", bufs=2) as sm,
            tc.tile_pool(name="scr", bufs=1) as scr,
            tc.tile_pool(name="pu", bufs=1, space="PSUM") as pup,
            tc.tile_pool(name="ptr", bufs=2, space="PSUM") as ptr,
            tc.tile_pool(name="psm", bufs=2, space="PSUM") as psm,
        ):
            # PSUM budget is 8 banks: the big psum tiles (pv, pu, pctx,
            # po) time-share one 4-bank slot via a common tag; the
            # scalar psums (pz/pr) share a 1-bank tag pair.
            BIG = "big"
            PSC = "psc"
            # ---------------- setup: constants -------------------------
            wenc_sb = const.tile([P, NFC, H], f32r)
            nc.gpsimd.dma_start(
                out=wenc_sb, in_=wenc.rearrange("(c p) h -> p c h", p=P)
            )
            identity = const.tile([P, P], f32)
            make_identity(nc, identity)
            ones_col = const.tile([P, 1], f32)
            nc.vector.memset(ones_col, 1.0)
            ones_row = const.tile([1, P], f32)
            nc.vector.memset(ones_row, 1.0)

            dhT = const.tile([P, NH, NB], f32r)
            for b in range(NB):
                nc.gpsimd.dma_start(
                    out=dhT[:, :, b], in_=dh[b].rearrange("(k p) -> p k", p=P)
                )
            benc_b4 = const.tile([NB, H], f32)
            nc.sync.dma_start(
                out=benc_b4, in_=benc_d.ap().partition_broadcast(NB)
            )
            bctx_b4 = const.tile([NB, H], f32)
            nc.sync.dma_start(
                out=bctx_b4, in_=bctx_d.ap().partition_broadcast(NB)
            )

            prod = scr.tile([P, F], f32)  # ttr elementwise-product sink

            # ---------------- setup: v = W_enc @ dh --------------------
            # Build W_enc^T one 128-row h-chunk at a time via PE
            # transposes (full fp32 bits), contract against dh^T into
            # psum_v with f32r matmuls.
            pv = pup.tile([P, F], f32, tag=BIG)
            for k in range(NH):
                wT = encp.tile([P, F], f32r, tag="enc")
                for jj in range(NJ):
                    pt = ptr.tile([P, NMM], f32)
                    for q in range(NMM // P):
                        j = jj * (NMM // P) + q
                        nc.tensor.transpose(
                            pt[:, q * P:(q + 1) * P],
                            wenc_sb[:, j, k * P:(k + 1) * P].bitcast(f32),
                            identity,
                        )
                    nc.scalar.copy(wT[:, jj * NMM:(jj + 1) * NMM], pt)
                for jj in range(NJ):
                    nc.tensor.matmul(
                        pv[0:NB, jj * NMM:(jj + 1) * NMM],
                        lhsT=dhT[:, k, :],
                        rhs=wT[:, jj * NMM:(jj + 1) * NMM],
                        start=(k == 0),
                        stop=(k == NH - 1),
                    )
            v_sb = const.tile([NB, F], f32)
            nc.scalar.copy(v_sb, pv[0:NB, :])

            # m-hat = ALPHA * ||v||  (per batch), shipped via DRAM for
            # partition-broadcast reload
            nrm2 = const.tile([NB, 1], f32)
            nc.vector.scalar_tensor_tensor(
                out=prod[0:NB, :], in0=v_sb, scalar=1.0, in1=v_sb,
                op0=ALU.mult, op1=ALU.mult, accum_out=nrm2,
            )
            nrm = const.tile([NB, 1], f32)
            nc.scalar.sqrt(nrm, nrm2)
            negm = const.tile([NB, 1], f32)
            nc.vector.tensor_scalar_mul(negm, nrm, -float(ALPHA))
            nc.sync.dma_start(out=v_scr.ap(), in_=v_sb)
            nc.sync.dma_start(out=nm_scr.ap(), in_=negm)

            negm_bc = const.tile([P, NB], f32)
            for b in range(NB):
                nc.gpsimd.dma_start(
                    out=negm_bc[:, b:b + 1],
                    in_=nm_scr.ap()[b].partition_broadcast(P),
                )

            # ---------------- streaming batches ------------------------
            for b in range(NB):
                vb = vbc.tile([P, F], f32, tag="vbc")
                nc.gpsimd.dma_start(
                    out=vb, in_=v_scr.ap()[b].partition_broadcast(P)
                )
                scores = sm.tile([P, NS], f32, tag="scores")
                wt = sm.tile([P, NS], f32r, tag="wt")
                pu_full = pup.tile([P, F], f32, tag=BIG)
                pu = pu_full[0:1, :]

                for c in range(NS):
                    enc_t = encp.tile([P, F], f32r, tag="enc")
                    nc.gpsimd.dma_start(
                        out=enc_t, in_=enc[b, c * P:(c + 1) * P, :]
                    )
                    nc.vector.scalar_tensor_tensor(
                        out=prod,
                        in0=enc_t.bitcast(f32),
                        scalar=1.0,
                        in1=vb,
                        op0=ALU.mult,
                        op1=ALU.mult,
                        accum_out=scores[:, c:c + 1],
                    )
                    nc.scalar.activation(
                        out=wt[:, c:c + 1],
                        in_=scores[:, c:c + 1],
                        func=ACTF.Exp,
                        bias=negm_bc[:, b:b + 1],
                        scale=1.0,
                    )
                    for j in range(NJ):
                        nc.tensor.matmul(
                            pu[:, j * NMM:(j + 1) * NMM],
                            lhsT=wt[:, c:c + 1],
                            rhs=enc_t[:, j * NMM:(j + 1) * NMM],
                            start=(c == 0),
                            stop=(c == NS - 1),
                        )

                # softmax denominator and normalization
                zcol = sm.tile([P, 1], f32, tag="zcol")
                nc.vector.tensor_reduce(
                    zcol, wt.bitcast(f32), axis=X, op=ALU.add
                )
                pz_full = psm.tile([P, 1], f32, tag=PSC)
                pz = pz_full[0:1, :]
                nc.tensor.matmul(pz, lhsT=zcol, rhs=ones_col, start=True, stop=True)
                z_sb = sm.tile([1, 1], f32, tag="z")
                nc.scalar.copy(z_sb, pz)
                r_sb = sm.tile([1, 1], f32, tag="r")
                nc.vector.reciprocal(r_sb, z_sb)
                pr = psm.tile([P, 1], f32, tag=PSC)
                nc.tensor.matmul(pr, lhsT=ones_row, rhs=r_sb, start=True, stop=True)
                r_bc = sm.tile([P, 1], f32, tag="rbc")
                nc.scalar.copy(r_bc, pr)

                wnorm = sm.tile([P, NS], f32, tag="wnorm")
                nc.vector.tensor_scalar(
                    out=wnorm, in0=wt.bitcast(f32), scalar1=r_bc,
                    scalar2=None, op0=ALU.mult,
                )
                nc.sync.dma_start(
                    out=attn_d.ap()[b].rearrange("(c p) -> p c", p=P),
                    in_=wnorm,
                )
                u_sb = sm.tile([1, F], f32r, tag="u")
                nc.scalar.activation(
                    out=u_sb, in_=pu, func=ACTF.Copy, bias=0.0, scale=r_sb
                )
                nc.sync.dma_start(out=u_scr.ap()[b:b + 1, :], in_=u_sb)

            # ---------------- tail: projections ------------------------
            wctx_tiles = []
            for i in range(NFC // 2):
                wc = encp.tile([P, 2, H], f32r, tag="enc")
                nc.gpsimd.dma_start(
                    out=wc,
                    in_=wctx[2 * i * P:(2 * i + 2) * P, :].rearrange(
                        "(cc p) h -> p cc h", p=P
                    ),
                )
                wctx_tiles.append(wc)

            U_all = const.tile([P, NFC, NB], f32r)
            for b in range(NB):
                nc.sync.dma_start(
                    out=U_all[:, :, b],
                    in_=u_scr.ap()[b].rearrange("(c p) -> p c", p=P),
                )
            pctx_full = pup.tile([P, F], f32, tag=BIG)
            pctx = pctx_full[0:NB, 0:H]
            for c in range(NFC):
                for jj in range(NHJ):
                    nc.tensor.matmul(
                        pctx[:, jj * NMH:(jj + 1) * NMH],
                        lhsT=U_all[:, c, :],
                        rhs=wenc_sb[:, c, jj * NMH:(jj + 1) * NMH],
                        start=(c == 0),
                        stop=(c == NFC - 1),
                    )
            ctx_sb = const.tile([NB, H], f32r)
            nc.vector.tensor_tensor(out=ctx_sb, in0=pctx, in1=benc_b4, op=ALU.add)
            nc.sync.dma_start(out=ctx_scr.ap(), in_=ctx_sb)

            combT = const.tile([P, NFC, NB], f32r)
            nc.vector.tensor_copy(out=combT[:, NH:NFC, :], in_=dhT)
            for b in range(NB):
                nc.sync.dma_start(
                    out=combT[:, 0:NH, b],
                    in_=ctx_scr.ap()[b].rearrange("(k p) -> p k", p=P),
                )
            po_full = pup.tile([P, F], f32, tag=BIG)
            po = po_full[0:NB, 0:H]
            for c in range(NFC):
                wc = wctx_tiles[c // 2]
                for jj in range(NHJ):
                    nc.tensor.matmul(
                        po[:, jj * NMH:(jj + 1) * NMH],
                        lhsT=combT[:, c, :],
                        rhs=wc[:, c % 2, jj * NMH:(jj + 1) * NMH],
                        start=(c == 0),
                        stop=(c == NFC - 1),
                    )
            pre = const.tile([NB, H], f32)
            nc.vector.tensor_tensor(out=pre, in0=po, in1=bctx_b4, op=ALU.add)
            out_sb = const.tile([NB, H], f32)
            nc.scalar.activation(out=out_sb, in_=pre, func=ACTF.Tanh)
            nc.sync.dma_start(out=out_d.ap(), in_=out_sb)

    nc.compile()
    return nc


def _get_nc(NB, S, F, H):
    key = (NB, S, F, H)
    if key not in _BUILD_CACHE:
        _BUILD_CACHE[key] = _build_nc(NB, S, F, H)
    return _BUILD_CACHE[key]


def _shard_inputs(encoder_outputs, decoder_hidden, W_enc, b_enc, W_ctx, b_ctx):
    B = encoder_outputs.shape[0]
    nb = B // NCORES
    dh = np.ascontiguousarray(decoder_hidden[0])  # (B, H)
    in_maps = []
    for i in range(NCORES):
        in_maps.append(
            {
                "enc": np.ascontiguousarray(
                    encoder_outputs[i * nb:(i + 1) * nb]
                ),
                "dh": np.ascontiguousarray(dh[i * nb:(i + 1) * nb]),
                "w_enc": np.ascontiguousarray(W_enc),
                "b_enc": np.ascontiguousarray(b_enc),
                "w_ctx": np.ascontiguousarray(W_ctx),
                "b_ctx": np.ascontiguousarray(b_ctx),
            }
        )
    return in_maps


def kernel(encoder_outputs, decoder_hidden, W_enc, b_enc, W_ctx, b_ctx):
    from concourse.bass_utils import run_bass_kernel_spmd

    encoder_outputs = np.asarray(encoder_outputs, np.float32)
    decoder_hidden = np.asarray(decoder_hidden, np.float32)
    W_enc = np.asarray(W_enc, np.float32)
    b_enc = np.asarray(b_enc, np.float32)
    W_ctx = np.asarray(W_ctx, np.float32)
    b_ctx = np.asarray(b_ctx, np.float32)

    B, S, F = encoder_outputs.shape
    H = decoder_hidden.shape[2]
    NB = B // NCORES

    nc = _get_nc(NB, S, F, H)
    in_maps = _shard_inputs(
        encoder_outputs, decoder_hidden, W_enc, b_enc, W_ctx, b_ctx
    )
    res = run_bass_kernel_spmd(nc, in_maps, list(range(NCORES)))
    out = np.concatenate([res.results[i]["out"] for i in range(NCORES)], axis=0)
    attn = np.concatenate(
        [res.results[i]["attn"] for i in range(NCORES)], axis=0
    )
    return (
        out[:, None, :].astype(np.float32),
        attn[:, :, None].astype(np.float32),
    )


# revision 17
# speedup vs baseline: 38.0349x; 38.0349x over previous
"""Trainium2 Bass kernel for nn_DotAttention (dense_transformer).

Reference computation (per batch b):
    proj   = enc_b @ W_enc + b_enc            # (S, H)
    scores = proj @ dh_b                      # (S,)
    w      = softmax(scores)                  # (S,)
    ctx    = proj^T @ w                       # (H,)
    out    = tanh([ctx; dh_b] @ W_ctx + b_ctx)

Algebraic restructuring (exact in real arithmetic):
    scores = enc_b @ (W_enc @ dh_b) + (b_enc . dh_b)
             -- the scalar offset cancels in softmax, so drop it
    ctx    = W_enc^T @ (enc_b^T @ w) + b_enc  -- since sum(w) == 1
This avoids materializing proj (275 GFLOP -> ~1.5 GFLOP) and makes the
kernel one streaming pass over encoder_outputs, fused per 128-row tile:
DVE scalar_tensor_tensor (fused multiply + row-reduce) for scores,
ACT exp, PE fp32r matmuls for the weighted sum.

Softmax max-trick uses a precomputed safe bound m = 3.9*||v||
(v = W_enc @ dh_b): scores are N(0, ||v||^2) for the given input
distribution, so the true max lies within [2.4, 6]*sigma with
overwhelming probability, keeping every exp/sum/reciprocal in normal
fp32 range; softmax is shift-invariant so the result is exact.

float32r (TF32-class PE path, 1 cycle/row vs 4 for fp32; measured HW
matmul relerr ~1.3e-4) is used for the weighted-sum pass and the tail
projections. The scores themselves are computed in full fp32 on DVE.

Sharding: data-parallel over batch B=32 across 8 cores (4 per core);
weights replicated.
"""

import numpy as np

P = 128           # SBUF partitions
NCORES = 8
ALPHA = 3.9       # softmax max-bound multiplier (see module docstring)

_BUILD_CACHE = {}


def _build_nc(NB, S, F, H, reps=1):
    """Build the per-core program. NB: batches per core; S: sequence;
    F = 2H: encoder feature dim; H: hidden dim."""
    import concourse.bacc as bacc
    import concourse.tile as tile
    from concourse import mybir
    from concourse.masks import make_identity

    f32 = mybir.dt.float32
    f32r = mybir.dt.float32r
    ALU = mybir.AluOpType
    ACTF = mybir.ActivationFunctionType
    X = mybir.AxisListType.X

    NS = S // P          # s-chunks per batch
    NFC = F // P         # f-chunks (128-wide)
    NH = H // P          # h-chunks (128-wide)
    NMM = min(512, F)    # psum bank width in fp32 (moving-dim chunk)
    NMH = min(512, H)    # tail column chunk
    NJ = F // NMM        # pass-2 column chunks
    NHJ = H // NMH       # tail column chunks
    assert NFC == 2 * NH

    nc = bacc.Bacc("TRN2", target_bir_lowering=False, debug=False)

    enc_d = nc.declare_dram_parameter("enc", [NB, S, F], f32r, isOutput=False)
    dh_d = nc.declare_dram_parameter("dh", [NB, H], f32, isOutput=False)
    wenc_d = nc.declare_dram_parameter("w_enc", [F, H], f32r, isOutput=False)
    benc_d = nc.declare_dram_parameter("b_enc", [H], f32, isOutput=False)
    wctx_d = nc.declare_dram_parameter("w_ctx", [F, H], f32r, isOutput=False)
    bctx_d = nc.declare_dram_parameter("b_ctx", [H], f32, isOutput=False)
    out_d = nc.declare_dram_parameter("out", [NB, H], f32, isOutput=True)
    attn_d = nc.declare_dram_parameter("attn", [P, NB, NS], f32, isOutput=True)

    # DRAM scratch for cheap layout changes (small)
    v_scr = nc.dram_tensor("v_scr", [NB, F], f32)

    enc = enc_d.ap()
    dh = dh_d.ap()
    wenc = wenc_d.ap()
    wctx = wctx_d.ap()

    with tile.TileContext(nc) as tc:
        with (
            tc.tile_pool(name="const", bufs=1) as const,
            tc.tile_pool(name="encp", bufs=8) as encp,
            tc.tile_pool(name="vbc", bufs=2) as vbc,
            tc.tile_pool(name="sm", bufs=2) as sm,
            tc.tile_pool(name="scr", bufs=1) as scr,
            tc.tile_pool(name="pu", bufs=1, space="PSUM") as pup,
            tc.tile_pool(name="ptr", bufs=2, space="PSUM") as ptr,
            tc.tile_pool(name="psm", bufs=2, space="PSUM") as psm,
        ):
            # PSUM budget is 8 banks: the big psum tiles (pv, pu, pctx,
            # po) time-share one 4-bank slot via a common tag; the
            # scalar psums (pz/pr) share a 1-bank tag pair.
            BIG = "big"
            PSC = "psc"
            for _rep in range(reps):
                # ---------------- setup: constants -------------------------
                wenc_sb = const.tile([P, NFC, H], f32r)
                nc.sync.dma_start(
                    out=wenc_sb, in_=wenc.rearrange("(c p) h -> p c h", p=P)
                )
                identity = const.tile([P, P], f32)
                make_identity(nc, identity)
                ones_col = const.tile([P, 1], f32)
                nc.vector.memset(ones_col, 1.0)
                ones_row = const.tile([1, P], f32)
                nc.vector.memset(ones_row, 1.0)

                dh_nat = const.tile([NB, H], f32)
                nc.sync.dma_start(out=dh_nat, in_=dh)
                dhT = const.tile([P, NH, NB], f32)
                for k in range(NH):
                    ptk = ptr.tile([P, NMM], f32, tag="pt")
                    nc.tensor.transpose(
                        ptk[:, 0:NB], dh_nat[:, k * P:(k + 1) * P],
                        identity[0:NB, 0:NB],
                    )
                    nc.scalar.copy(dhT[:, k, :], ptk[:, 0:NB])
                benc_b4 = const.tile([NB, H], f32)
                nc.sync.dma_start(
                    out=benc_b4, in_=benc_d.ap().partition_broadcast(NB)
                )
                bctx_b4 = const.tile([NB, H], f32)
                nc.sync.dma_start(
                    out=bctx_b4, in_=bctx_d.ap().partition_broadcast(NB)
                )

                prod = scr.tile([P, F], f32)  # ttr elementwise-product sink

                # ---------------- setup: v = W_enc @ dh --------------------
                # Build W_enc^T one 128-row h-chunk at a time via PE
                # transposes (full fp32 bits), contract against dh^T in
                # full fp32 (v feeds the softmax logits).
                pv = pup.tile([P, F], f32, tag=BIG)
                for k in range(NH):
                    wT = encp.tile([P, F], f32, tag="enc")
                    for jj in range(NJ):
                        pt = ptr.tile([P, NMM], f32)
                        for q in range(NMM // P):
                            j = jj * (NMM // P) + q
                            nc.tensor.transpose(
                                pt[:, q * P:(q + 1) * P],
                                wenc_sb[:, j, k * P:(k + 1) * P].bitcast(f32),
                                identity,
                            )
                        nc.scalar.copy(wT[:, jj * NMM:(jj + 1) * NMM], pt)
                    for jj in range(NJ):
                        nc.tensor.matmul(
                            pv[0:NB, jj * NMM:(jj + 1) * NMM],
                            lhsT=dhT[:, k, :],
                            rhs=wT[:, jj * NMM:(jj + 1) * NMM],
                            start=(k == 0),
                            stop=(k == NH - 1),
                        )
                v_sb = const.tile([NB, F], f32)
                nc.scalar.copy(v_sb, pv[0:NB, :])

                # m-hat = ALPHA * ||v||  (per batch), shipped via DRAM for
                # partition-broadcast reload
                nrm2 = const.tile([NB, 1], f32)
                nc.vector.scalar_tensor_tensor(
                    out=prod[0:NB, :], in0=v_sb, scalar=1.0, in1=v_sb,
                    op0=ALU.mult, op1=ALU.mult, accum_out=nrm2,
                )
                nrm = const.tile([NB, 1], f32)
                nc.scalar.sqrt(nrm, nrm2)
                negm = const.tile([NB, 1], f32)
                nc.vector.tensor_scalar_mul(negm, nrm, -float(ALPHA))
                nc.sync.dma_start(out=v_scr.ap(), in_=v_sb)

                ptn = ptr.tile([P, NMM], f32, tag="pt")
                nc.tensor.transpose(
                    ptn[0:1, 0:NB], negm, identity[0:NB, 0:NB]
                )
                negm_row = const.tile([1, NB], f32)
                nc.scalar.copy(negm_row, ptn[0:1, 0:NB])
                ptn2 = ptr.tile([P, NMM], f32, tag="pt")
                nc.tensor.matmul(
                    ptn2[:, 0:NB], lhsT=ones_row, rhs=negm_row,
                    start=True, stop=True,
                )
                negm_bc = const.tile([P, NB], f32)
                nc.scalar.copy(negm_bc, ptn2[:, 0:NB])

                # ---------------- streaming batches ------------------------
                wnorm_all = sm.tile([P, NB, NS], f32, tag="wnorm")
                U_all = sm.tile([P, NFC, NB], f32r, tag="uall")
                for b in range(NB):
                    vb = vbc.tile([P, F], f32, tag="vbc")
                    nc.gpsimd.dma_start(
                        out=vb, in_=v_scr.ap()[b].partition_broadcast(P)
                    )
                    scores = sm.tile([P, NS], f32, tag="scores")
                    wt = sm.tile([P, NS], f32r, tag="wt")
                    pu_full = pup.tile([P, F], f32, tag=BIG)
                    pu = pu_full[0:1, :]

                    for c in range(NS):
                        enc_t = encp.tile([P, F], f32r, tag="enc")
                        nc.sync.dma_start(
                            out=enc_t, in_=enc[b, c * P:(c + 1) * P, :]
                        )
                        nc.vector.scalar_tensor_tensor(
                            out=prod,
                            in0=enc_t.bitcast(f32),
                            scalar=1.0,
                            in1=vb,
                            op0=ALU.mult,
                            op1=ALU.mult,
                            accum_out=scores[:, c:c + 1],
                        )
                        nc.scalar.activation(
                            out=wt[:, c:c + 1],
                            in_=scores[:, c:c + 1],
                            func=ACTF.Exp,
                            bias=negm_bc[:, b:b + 1],
                            scale=1.0,
                        )
                        for j in range(NJ):
                            nc.tensor.matmul(
                                pu[:, j * NMM:(j + 1) * NMM],
                                lhsT=wt[:, c:c + 1],
                                rhs=enc_t[:, j * NMM:(j + 1) * NMM],
                                start=(c == 0),
                                stop=(c == NS - 1),
                            )

                    # softmax denominator and normalization
                    zcol = sm.tile([P, 1], f32, tag="zcol")
                    nc.vector.tensor_reduce(
                        zcol, wt.bitcast(f32), axis=X, op=ALU.add
                    )
                    pz_full = psm.tile([P, 1], f32, tag=PSC)
                    pz = pz_full[0:1, :]
                    nc.tensor.matmul(pz, lhsT=zcol, rhs=ones_col, start=True, stop=True)
                    z_sb = sm.tile([1, 1], f32, tag="z")
                    nc.scalar.copy(z_sb, pz)
                    r_sb = sm.tile([1, 1], f32, tag="r")
                    nc.vector.reciprocal(r_sb, z_sb)
                    pr = psm.tile([P, 1], f32, tag=PSC)
                    nc.tensor.matmul(pr, lhsT=ones_row, rhs=r_sb, start=True, stop=True)
                    r_bc = sm.tile([P, 1], f32, tag="rbc")
                    nc.scalar.copy(r_bc, pr)

                    nc.vector.tensor_scalar(
                        out=wnorm_all[:, b, :], in0=wt.bitcast(f32),
                        scalar1=r_bc, scalar2=None, op0=ALU.mult,
                    )
                    u_sb = sm.tile([1, F], f32, tag="u")
                    nc.scalar.activation(
                        out=u_sb, in_=pu, func=ACTF.Copy, bias=0.0, scale=r_sb
                    )
                    ptu = ptr.tile([P, NMM], f32, tag="pt")
                    for c in range(NFC):
                        nc.tensor.transpose(
                            ptu[:, c:c + 1],
                            u_sb[0:1, c * P:(c + 1) * P],
                            identity[0:1, 0:1],
                        )
                    nc.scalar.copy(U_all[:, :, b], ptu[:, 0:NFC])

                nc.sync.dma_start(out=attn_d.ap(), in_=wnorm_all)

                # ---------------- tail: projections ------------------------
                wctx_tiles = []
                for i in range(NFC // 2):
                    wc = encp.tile([P, 2, H], f32r, tag="enc")
                    nc.sync.dma_start(
                        out=wc,
                        in_=wctx[2 * i * P:(2 * i + 2) * P, :].rearrange(
                            "(cc p) h -> p cc h", p=P
                        ),
                    )
                    wctx_tiles.append(wc)

                pctx_full = pup.tile([P, F], f32, tag=BIG)
                pctx = pctx_full[0:NB, 0:H]
                for c in range(NFC):
                    for jj in range(NHJ):
                        nc.tensor.matmul(
                            pctx[:, jj * NMH:(jj + 1) * NMH],
                            lhsT=U_all[:, c, :],
                            rhs=wenc_sb[:, c, jj * NMH:(jj + 1) * NMH],
                            start=(c == 0),
                            stop=(c == NFC - 1),
                        )
                ctx_sb = const.tile([NB, H], f32)
                nc.vector.tensor_tensor(out=ctx_sb, in0=pctx, in1=benc_b4, op=ALU.add)

                combT = const.tile([P, NFC, NB], f32r)
                nc.vector.tensor_copy(out=combT[:, NH:NFC, :], in_=dhT)
                for k in range(NH):
                    ptc = ptr.tile([P, NMM], f32, tag="pt")
                    nc.tensor.transpose(
                        ptc[:, 0:NB], ctx_sb[:, k * P:(k + 1) * P],
                        identity[0:NB, 0:NB],
                    )
                    nc.scalar.copy(combT[:, k, :], ptc[:, 0:NB])
                po_full = pup.tile([P, F], f32, tag=BIG)
                po = po_full[0:NB, 0:H]
                for c in range(NFC):
                    wc = wctx_tiles[c // 2]
                    for jj in range(NHJ):
                        nc.tensor.matmul(
                            po[:, jj * NMH:(jj + 1) * NMH],
                            lhsT=combT[:, c, :],
                            rhs=wc[:, c % 2, jj * NMH:(jj + 1) * NMH],
                            start=(c == 0),
                            stop=(c == NFC - 1),
                        )
                pre = const.tile([NB, H], f32)
                nc.vector.tensor_tensor(out=pre, in0=po, in1=bctx_b4, op=ALU.add)
                out_sb = const.tile([NB, H], f32)
                nc.scalar.activation(out=out_sb, in_=pre, func=ACTF.Tanh)
                nc.sync.dma_start(out=out_d.ap(), in_=out_sb)

    nc.compile()
    return nc


def _get_nc(NB, S, F, H, reps=1):
    key = (NB, S, F, H, reps)
    if key not in _BUILD_CACHE:
        _BUILD_CACHE[key] = _build_nc(NB, S, F, H, reps)
    return _BUILD_CACHE[key]


def _shard_inputs(encoder_outputs, decoder_hidden, W_enc, b_enc, W_ctx, b_ctx):
    B = encoder_outputs.shape[0]
    nb = B // NCORES
    dh = np.ascontiguousarray(decoder_hidden[0])  # (B, H)
    in_maps = []
    for i in range(NCORES):
        in_maps.append(
            {
                "enc": np.ascontiguousarray(
                    encoder_outputs[i * nb:(i + 1) * nb]
                ),
                "dh": np.ascontiguousarray(dh[i * nb:(i + 1) * nb]),
                "w_enc": np.ascontiguousarray(W_enc),
                "b_enc": np.ascontiguousarray(b_enc),
                "w_ctx": np.ascontiguousarray(W_ctx),
                "b_ctx": np.ascontiguousarray(b_ctx),
            }
        )
    return in_maps


def kernel(encoder_outputs, decoder_hidden, W_enc, b_enc, W_ctx, b_ctx):
    from concourse.bass_utils import run_bass_kernel_spmd

    encoder_outputs = np.asarray(encoder_outputs, np.float32)
    decoder_hidden = np.asarray(decoder_hidden, np.float32)
    W_enc = np.asarray(W_enc, np.float32)
    b_enc = np.asarray(b_enc, np.float32)
    W_ctx = np.asarray(W_ctx, np.float32)
    b_ctx = np.asarray(b_ctx, np.float32)

    B, S, F = encoder_outputs.shape
    H = decoder_hidden.shape[2]
    NB = B // NCORES

    nc = _get_nc(NB, S, F, H)
    in_maps = _shard_inputs(
        encoder_outputs, decoder_hidden, W_enc, b_enc, W_ctx, b_ctx
    )
    res = run_bass_kernel_spmd(nc, in_maps, list(range(NCORES)))
    out = np.concatenate([res.results[i]["out"] for i in range(NCORES)], axis=0)
    attn = np.concatenate(
        [
            res.results[i]["attn"].transpose(1, 2, 0).reshape(NB, S)
            for i in range(NCORES)
        ],
        axis=0,
    )
    return (
        out[:, None, :].astype(np.float32),
        attn[:, :, None].astype(np.float32),
    )

